# revision 9
# baseline (speedup 1.0000x reference)
"""DressedQuantumNet on 8 TRN2 NeuronCores (pure data parallel).

Math: pre-net angles th = X @ pre_w.T (+ pre_b).  After H + RY(th') the
4-qubit state is the REAL product state

  psi = kron_w [cos(th'_w/2 + pi/4), sin(th'_w/2 + pi/4)],  th' = th + pre_b

and the rest of the circuit is a FIXED unitary V (depends only on
q_weights).  The per-wire bias rotation R(pre_b_w/2) is absorbed into
V' = V @ kron_w R(pre_b_w/2), so the device only needs raw th.  With
Wz[i,c] = sum_w post_w[c,w] z_w(i) + post_b[c] (sum_i probs_i == 1), the
whole head collapses to two real symmetric quadratic forms

  out_c = psi^T K_c psi,   K_c = Re(V'^H diag(Wz[:,c]) V')   [16 x 16]

Device pipeline per 8-row-tile slab (batch on SBUF partitions), software
pipelined A(s) | B(s-1) | C(s-2) to keep all engine FIFOs unblocked:

  A: DMA fp16 X^T slab (1 MiB, sync queue; consts preloaded on same queue)
     PE  4 accumulating matmuls/tile -> th in PSUM [128,t,4]
     ACT cs = Sin(+-0.5*th + pi/4)  -> (cos,sin) fp16
     DVE psi = (c0,s0)x(c1,s1)x(c2,s2)x(c3,s3)  [128, t, 16] fp16
  B: PE  transpose psi -> psiT [16t, 128] PSUM; ACT copy -> SBUF
     PE  qq_c = blockdiag(K_c) @ psiT  (c=0,1 -> one PSUM tile)
  C: DVE pq_c = psiT * qq_c  (SBUF x PSUM -> fp16)
     PE  z[t, c, p] = column-sum over the 16 states (selection matmul)
     ACT copy z -> resall; one DMA out at the end

Everything sits under the fp16 input-stream DMA floor (~8.4 MiB/core).
"""

from contextlib import ExitStack

import numpy as np

import concourse.bass as bass
import concourse.bacc as bacc_mod
import concourse.mybir as mybir
from concourse.bass_utils import run_bass_kernel_spmd
from concourse.tile import TileContext

N_CORES = 8
B_TOTAL = 65536
F_IN = 512
ROWS = B_TOTAL // N_CORES   # 8192 rows per core
P = 128
N_TILES = ROWS // P         # 64 row-tiles
G = 8                       # row-tiles per slab (1 MiB fp16 DMA)
N_SLABS = N_TILES // G      # 8

F32 = mybir.dt.float32
FP16 = mybir.dt.float16
PI = float(np.pi)

N_QUBITS, VAR_DEPTH = 4, 3


# ----------------------------------------------------------------- host math
def _gate_1q(g, w):
    ops = [np.eye(2, dtype=complex)] * N_QUBITS
    ops[w] = g
    U = ops[0]
    for i in range(1, N_QUBITS):
        U = np.kron(U, ops[i])
    return U


def _bit(i, w):  # wire 0 = most significant
    return (i >> (N_QUBITS - 1 - w)) & 1


def _cnot(c, t):
    M = np.zeros((16, 16), dtype=complex)
    for i in range(16):
        j = i ^ (1 << (N_QUBITS - 1 - t)) if _bit(i, c) else i
        M[j, i] = 1.0
    return M


def _ry(theta):
    c, s = np.cos(theta / 2), np.sin(theta / 2)
    return np.array([[c, -s], [s, c]], dtype=complex)


def _rz(theta):
    ph = np.exp(1j * theta / 2)
    return np.array([[np.conj(ph), 0], [0, ph]], dtype=complex)


def _fixed_unitary(qw):
    V = np.eye(16, dtype=complex)

    def app(Gm):
        nonlocal V
        V = Gm @ V

    def entangle():
        app(_cnot(0, 1)); app(_cnot(2, 3)); app(_cnot(1, 2))

    for k in range(VAR_DEPTH):
        entangle()
        for w in range(N_QUBITS):
            app(_gate_1q(_ry(qw[k, w]), w))
        for w in range(N_QUBITS):
            app(_gate_1q(_rz(qw[k, w]), w))
    for k in range(VAR_DEPTH):
        entangle()
        for w in range(N_QUBITS):
            app(_gate_1q(_ry(qw[k, w]), w))
        for w in range(N_QUBITS):
            app(_gate_1q(_rz(qw[3 + k, w]), w))
    entangle()
    return V


def _host_consts(pre_w, pre_b, q_weights, post_w, post_b):
    pre_w = np.asarray(pre_w, dtype=np.float64)
    pre_b = np.asarray(pre_b, dtype=np.float64)
    post_w = np.asarray(post_w, dtype=np.float64)
    post_b = np.asarray(post_b, dtype=np.float64)

    # whl[p, 4k + w] = pre_w[w, 128k + p]
    whl = np.zeros((P, 16), dtype=np.float16)
    for k in range(4):
        whl[:, 4 * k:4 * k + 4] = pre_w.T[P * k:P * (k + 1)].astype(np.float16)

    V = _fixed_unitary(np.asarray(q_weights, dtype=np.float64))
    R = np.eye(1)
    for w in range(N_QUBITS):
        d = pre_b[w] / 2.0
        R = np.kron(R, np.array([[np.cos(d), -np.sin(d)],
                                 [np.sin(d), np.cos(d)]]))
    Vp = V @ R

    # Wz[i, c] = sum_w post_w[c,w] z_w(i) + post_b[c]  (sum_i probs_i == 1)
    Wz = np.zeros((16, 2))
    for c in range(2):
        for i in range(16):
            Wz[i, c] = sum(
                post_w[c, w] * (1.0 - 2.0 * _bit(i, w)) for w in range(N_QUBITS)
            ) + post_b[c]

    # K_c = Re(V'^H diag(Wz_c) V')  -- real symmetric 16x16; block-diagonal
    kb = []
    for c in range(2):
        Kc = (Vp.conj().T @ np.diag(Wz[:, c]) @ Vp).real
        blk = np.zeros((P, P), dtype=np.float16)
        for t in range(G):
            blk[16 * t:16 * t + 16, 16 * t:16 * t + 16] = Kc.T.astype(np.float16)
        kb.append(blk)

    selz = np.zeros((P, G), dtype=np.float16)
    for t in range(G):
        selz[16 * t:16 * t + 16, t] = 1.0

    ident = np.eye(P, dtype=np.float16)
    return {
        "whl": np.ascontiguousarray(whl),
        "k0b": np.ascontiguousarray(kb[0]),
        "k1b": np.ascontiguousarray(kb[1]),
        "selz": np.ascontiguousarray(selz),
        "ident": np.ascontiguousarray(ident),
    }


# ------------------------------------------------------------- device kernel
def build_bass(rows=ROWS):
    n_tiles = rows // P
    n_slabs = n_tiles // G
    assert n_slabs * G == n_tiles

    nc = bacc_mod.Bacc(None, target_bir_lowering=False)
    # host-packed flat: concatenation of per-slab [P, 4, G*P] fp16 blocks
    ht_d = nc.dram_tensor("htp", [rows * 4 * P], FP16, kind="ExternalInput")
    whl_d = nc.dram_tensor("whl", [P, 16], FP16, kind="ExternalInput")
    k0_d = nc.dram_tensor("k0b", [P, P], FP16, kind="ExternalInput")
    k1_d = nc.dram_tensor("k1b", [P, P], FP16, kind="ExternalInput")
    sz_d = nc.dram_tensor("selz", [P, G], FP16, kind="ExternalInput")
    id_d = nc.dram_tensor("ident", [P, P], FP16, kind="ExternalInput")
    # out_dev[t, c, s, p] = out[(s*G + t)*128 + p, c]; host unscrambles
    out_d = nc.dram_tensor("out", [G, 2, n_slabs, P], F32, kind="ExternalOutput")

    with TileContext(nc) as tc, ExitStack() as ctx:
        const = ctx.enter_context(tc.tile_pool(name="const", bufs=1))
        whl = const.tile([P, 16], FP16)
        nc.sync.dma_start(whl, whl_d[:])
        k0b = const.tile([P, P], FP16)
        nc.sync.dma_start(k0b, k0_d[:])
        k1b = const.tile([P, P], FP16)
        nc.sync.dma_start(k1b, k1_d[:])
        selz = const.tile([P, G], FP16)
        nc.sync.dma_start(selz, sz_d[:])
        ident = const.tile([P, P], FP16)
        nc.sync.dma_start(ident, id_d[:])
        pi4 = const.tile([P, 1], F32)
        nc.vector.memset(pi4, PI / 4)

        xp = ctx.enter_context(tc.tile_pool(name="xin", bufs=n_slabs))
        angp = ctx.enter_context(tc.tile_pool(name="angp", bufs=2, space="PSUM"))
        csp = ctx.enter_context(tc.tile_pool(name="csp", bufs=3))
        pp = ctx.enter_context(tc.tile_pool(name="pp", bufs=3))
        psip = ctx.enter_context(tc.tile_pool(name="psip", bufs=4))
        ptp = ctx.enter_context(tc.tile_pool(name="ptp", bufs=2, space="PSUM"))
        pts = ctx.enter_context(tc.tile_pool(name="pts", bufs=4))
        qqp = ctx.enter_context(tc.tile_pool(name="qqp", bufs=2, space="PSUM"))
        prp = ctx.enter_context(tc.tile_pool(name="prp", bufs=3))
        zp = ctx.enter_context(tc.tile_pool(name="zp", bufs=2, space="PSUM"))
        rp = ctx.enter_context(tc.tile_pool(name="res", bufs=1))

        resall = rp.tile([G, 2, n_slabs, P], F32)

        def stage_a(s):
            gb = G * P
            base = s * P * 4 * gb
            ht = xp.tile([P, 4, gb], FP16, tag="ht")
            nc.sync.dma_start(
                ht,
                ht_d[base:base + P * 4 * gb].rearrange(
                    "(p k b) -> p k b", p=P, k=4),
            )
            # th[p, t, w] in PSUM, fp32
            ang = angp.tile([P, G, 4], F32)
            for t in range(G):
                bs = t * P
                for k in range(4):
                    nc.tensor.matmul(
                        ang[:, t, :],
                        ht[:, k, bs:bs + P],
                        whl[:, 4 * k:4 * k + 4],
                        start=(k == 0), stop=(k == 3),
                    )
            # cs[p, t, w, 0] = cos(th/2 + pi/4) = Sin(-.5*th + pi/4)
            # cs[p, t, w, 1] = sin(th/2 + pi/4) = Sin(+.5*th + pi/4)
            cs = csp.tile([P, G, 4, 2], FP16, tag="cs")
            nc.scalar.activation(
                cs[:, :, :, 0], ang, mybir.ActivationFunctionType.Sin,
                bias=pi4, scale=-0.5,
            )
            nc.scalar.activation(
                cs[:, :, :, 1], ang, mybir.ActivationFunctionType.Sin,
                bias=pi4, scale=0.5,
            )
            # psi = kron of the four (c,s) pairs -> [P, G, 4, 4] fp16
            p01 = pp.tile([P, G, 2, 2], FP16, tag="p01")
            nc.vector.tensor_mul(
                p01,
                cs[:, :, 0, :].unsqueeze(3).broadcast_to([P, G, 2, 2]),
                cs[:, :, 1, :].unsqueeze(2).broadcast_to([P, G, 2, 2]),
            )
            p23 = pp.tile([P, G, 2, 2], FP16, tag="p23")
            nc.vector.tensor_mul(
                p23,
                cs[:, :, 2, :].unsqueeze(3).broadcast_to([P, G, 2, 2]),
                cs[:, :, 3, :].unsqueeze(2).broadcast_to([P, G, 2, 2]),
            )
            psi = psip.tile([P, G, 4, 4], FP16, tag="psi")
            nc.vector.tensor_mul(
                psi,
                p01.rearrange("p g a b -> p g (a b)")
                   .unsqueeze(3).broadcast_to([P, G, 4, 4]),
                p23.rearrange("p g a b -> p g (a b)")
                   .unsqueeze(2).broadcast_to([P, G, 4, 4]),
            )
            return psi

        def stage_b(s, psi):
            # psiT[16t + i, p] via PE transpose, then to SBUF
            psiT = ptp.tile([16 * G, P], FP16)
            nc.tensor.transpose(
                psiT, psi.rearrange("p g a b -> p (g a b)"), ident
            )
            psiTs = pts.tile([16 * G, P], FP16, tag="psiTs")
            nc.scalar.copy(psiTs, psiT)
            # qq_c = blockdiag(K_c) @ psiT
            qq = qqp.tile([16 * G, 2, P], F32, tag="qq")
            nc.tensor.matmul(qq[:, 0, :], k0b, psiTs, start=True, stop=True)
            nc.tensor.matmul(qq[:, 1, :], k1b, psiTs, start=True, stop=True)
            return psiTs, qq

        def stage_c(s, psiTs, qq):
            # pq_c = psiT * qq_c  (SBUF x PSUM)
            pq = prp.tile([16 * G, 2, P], FP16, tag="pq")
            nc.vector.tensor_mul(pq[:, 0, :], qq[:, 0, :], psiTs)
            nc.vector.tensor_mul(pq[:, 1, :], qq[:, 1, :], psiTs)
            # z[t, c, p] = sum_i pq[16t + i, c, p]
            z_ps = zp.tile([G, 2, P], F32)
            nc.tensor.matmul(z_ps, selz, pq, start=True, stop=True)
            nc.scalar.copy(resall[:, :, s, :], z_ps)

        live = {}
        for s in range(n_slabs + 2):
            if s < n_slabs:
                live[s] = [stage_a(s)]
            if 1 <= s and s - 1 < n_slabs:
                psiTs, qq = stage_b(s - 1, live[s - 1][0])
                live[s - 1] += [psiTs, qq]
            if 2 <= s:
                _, psiTs, qq = live.pop(s - 2)
                stage_c(s - 2, psiTs, qq)

        nc.sync.dma_start(out_d[:], resall)

    nc.finalize()
    return nc


_NC_CACHE = {}


def _get_nc(rows=ROWS):
    if rows not in _NC_CACHE:
        _NC_CACHE[rows] = build_bass(rows=rows)
    return _NC_CACHE[rows]


def _pack_input(x):
    """x [ROWS, F] f32 -> flat fp16: per-slab [P, 4, G*P] packs,
    pack[p, k, b] = x[slab_row0 + b, 128*k + p]."""
    rows = x.shape[0]
    h = x.astype(np.float16)
    parts = []
    r0 = 0
    for _ in range(rows // (G * P)):
        gb = G * P
        blk = h[r0:r0 + gb].reshape(gb, 4, P).transpose(2, 1, 0)
        parts.append(np.ascontiguousarray(blk).reshape(-1))
        r0 += gb
    return np.concatenate(parts)


def run(input_features, pre_w, pre_b, q_weights, post_w, post_b, **spmd_kwargs):
    x = np.asarray(input_features, dtype=np.float32)
    assert x.shape == (B_TOTAL, F_IN), x.shape
    consts = _host_consts(pre_w, pre_b, q_weights, post_w, post_b)
    in_maps = []
    for c in range(N_CORES):
        ht = _pack_input(x[c * ROWS:(c + 1) * ROWS])
        in_maps.append(dict(consts, htp=ht))
    nc = _get_nc()
    r = run_bass_kernel_spmd(nc, in_maps, core_ids=list(range(N_CORES)), **spmd_kwargs)
    # out_dev[t, c, s, p] -> out[(s*G + t)*128 + p, c]
    outs = []
    for c in range(N_CORES):
        o = r.results[c]["out"]                             # [t, c, s, p]
        o = o.transpose(2, 0, 3, 1).reshape(ROWS, 2)        # [s, t, p, c]
        outs.append(o)
    out = np.concatenate(outs, axis=0)
    return out.astype(np.float32), r


def kernel(input_features, pre_w, pre_b, q_weights, post_w, post_b):
    out, _ = run(input_features, pre_w, pre_b, q_weights, post_w, post_b)
    return out
